# revision 25
# baseline (speedup 1.0000x reference)
"""Trainium2 Bass kernel for nn_ExpertAdaRMSLayer (AdaRMS transformer layer).

Sharding: 8 cores = 4 batches (DP) x 2 token-halves. Each core computes its
1024 tokens end-to-end with no collectives; k/v (nkv=1) are computed
redundantly by the pair of cores sharing a batch. All activations are kept
feature-major [feature, token] on device; the host pre-transposes inputs /
weights and re-assembles the output. Columns are rolled per core so "own"
tokens are always columns 0..1023 (keeps the SPMD program uniform; attention
is permutation-invariant over keys).

v2 (vs v1):
  - The adaptive gains g = w*(1 + temb @ tw.T) are computed on the host
    (tiny per-batch matvec) and passed as inputs; the 512 N=1 matmuls and
    the tw1/tw2 weight streams are gone.
  - Row->all-partition broadcasts (1/rms, 1/denom) go through a K=1 f32r
    matmul into PSUM instead of a DRAM DMA bounce (removes DMA latency from
    the softmax/rms critical paths).
  - Stage B (rms1) is interleaved with the q-projection so the PE isn't
    idle while the scalar engine runs Square/Sqrt chains.
  - Stage E runs nt-major so rms2 of the first token tile overlaps the
    o-projection matmuls of the second.
"""

import os
import sys
from contextlib import ExitStack

import numpy as np

sys.path.insert(0, "/opt/trn_rl_repo")

import ml_dtypes

import concourse.bass as bass
import concourse.mybir as mybir
import concourse.tile as tile

BF16 = ml_dtypes.bfloat16
F32 = np.float32

# Model dims (hardcoded per spec)
HIDDEN, NQ, NKV, HD, INTER = 2048, 8, 1, 256, 8192
B, S = 4, 2048
EPS = 1e-6
ROPE_BASE = 10000.0

P = 128
HC = HIDDEN // P          # 16 hidden chunks
IC = INTER // P           # 64 inter chunks
QC = (NQ * HD) // P       # 16 q-feature chunks
KC_HD = HD // P           # 2 head-dim chunks
T_OWN = S // 2            # 1024 own tokens per core
T_FULL = S                # 2048 tokens per batch
FD = 512                  # matmul free-dim tile (one PSUM bank of f32)
NT_OWN = T_OWN // FD      # 2
NT_FULL = T_FULL // FD    # 4
N_CORES = 8

DT = mybir.dt.bfloat16    # matmul operand dtype
DT_R = mybir.dt.float32r  # full-rate fp32 dtype for rms sum-of-squares
AF = mybir.ActivationFunctionType
ALU = mybir.AluOpType

_CACHE = {}
LAST_RESULTS = None


PADW = 132  # padded strip row length: keeps strip DMAs off the 1-wait
             # DIRECT2D path (3 unmergeable src dims -> generic DMA)


def _strips(WT, KC, MC):
    """WT: [K, M] f32 with rows = contraction dim. Returns bf16 array
    [MC, 128, KC, PADW] with [m][p][kc][:128] = WT[kc*128+p, m*128+j]."""
    K, M = WT.shape
    assert K == KC * P and M == MC * P
    A = WT.reshape(KC, P, MC, P).transpose(2, 1, 0, 3)
    out = np.zeros((MC, P, KC, PADW), dtype=BF16)
    out[:, :, :, :P] = A.astype(BF16)
    return out


def build_program():
    if "nc" in _CACHE:
        return _CACHE["nc"]

    nc = bass.Bass()
    dram = {}

    def inp(name, shape, dt):
        dram[name] = nc.declare_dram_parameter(name, list(shape), dt,
                                               isOutput=False)

    inp("xT", (HIDDEN, T_FULL), mybir.dt.float32)
    inp("cosT", (P, T_FULL), DT)
    inp("sinT", (P, T_FULL), DT)
    inp("g1", (P, HC), mybir.dt.float32)   # host: w1*(1+temb@tw1.T)
    inp("g2", (P, HC), mybir.dt.float32)
    inp("wq", (QC, P, HC, PADW), DT)
    inp("wk", (KC_HD, P, HC, PADW), DT)
    inp("wv", (HC, P, HD), DT)
    inp("wo", (HC, P, QC, PADW), DT)
    inp("wg", (IC, P, HC, PADW), DT)
    inp("wu", (IC, P, HC, PADW), DT)
    inp("wd", (HC, P, IC, PADW), DT)
    outT = nc.declare_dram_parameter("outT", [HIDDEN, T_OWN],
                                     mybir.dt.float32, isOutput=True)
    res2T = nc.dram_tensor("res2T", [HIDDEN, T_OWN], mybir.dt.float32)

    _build_kernel(nc, dram, outT, res2T)
    if not os.environ.get("KERNEL_NO_WAIT_SPLIT"):
        _split_dma_waits(nc)
    _CACHE["nc"] = nc
    return nc


def _split_dma_waits(nc):
    """This walrus encodes at most ONE sync-wait per instruction (the ISA
    EVENTS struct has a single wait slot and this build refuses to split).
    Hoist all waits of multi-wait instructions onto standalone
    event-semaphore instructions on the issuing engine/sequencer, which
    executes them in program order before the original instruction."""
    n = 0
    for f in nc.m.functions:
        for bb in f.blocks:
            out = []
            changed = False
            for inst in bb.instructions:
                si = inst.sync_info
                if si is not None and len(si.on_wait) > 1:
                    for w in si.on_wait:
                        ev = mybir.InstEventSemaphore(
                            name=f"{inst.name}_w{n}", ins=[], outs=[])
                        ev.engine = inst.engine
                        ev.sync_info = mybir.SyncInfo(on_wait=[w],
                                                      on_update=[])
                        out.append(ev)
                        n += 1
                    inst.sync_info = mybir.SyncInfo(
                        on_wait=[], on_update=list(si.on_update))
                    changed = True
                out.append(inst)
            if changed:
                bb.instructions[:] = out
    return n


def _build_kernel(nc, dram, outT, res2T):
    xT_v = dram["xT"][:, :].rearrange("(c p) t -> p c t", p=P)
    res2T_v = res2T[:, :].rearrange("(c p) t -> p c t", p=P)
    outT_v = outT[:, :].rearrange("(c p) t -> p c t", p=P)

    with tile.TileContext(nc) as tc, ExitStack() as top:
        const = top.enter_context(tc.tile_pool(name="const", bufs=1))
        psA = top.enter_context(tc.tile_pool(name="psA", bufs=6, space="PSUM"))
        psB = top.enter_context(tc.tile_pool(name="psB", bufs=2, space="PSUM"))
        rowp = top.enter_context(tc.tile_pool(name="rowr", bufs=2))

        ones_bf = const.tile([P, 1], DT)
        nc.vector.memset(ones_bf, 1.0)
        ones_rf = const.tile([P, 1], mybir.dt.float32, name="ones_rf")
        nc.vector.memset(ones_rf, 1.0)
        ones_r = ones_rf.bitcast(DT_R)
        ones1f = const.tile([1, P], mybir.dt.float32, name="ones1f")
        nc.vector.memset(ones1f, 1.0)
        ones1r = ones1f.bitcast(DT_R)
        g1 = const.tile([P, HC], mybir.dt.float32, name="g1")
        nc.gpsimd.dma_start(out=g1, in_=dram["g1"][:, :])
        g2 = const.tile([P, HC], mybir.dt.float32, name="g2")
        nc.gpsimd.dma_start(out=g2, in_=dram["g2"][:, :])
        eps_t = const.tile([1, 1], mybir.dt.float32, name="eps_t")
        nc.vector.memset(eps_t, EPS)

        def bcast_psum(row_f32, width, sbuf_tile):
            """Broadcast [1, width] f32 row across 128 partitions via a K=1
            f32r matmul into PSUM, then copy to SBUF (no DRAM bounce). The
            row is first copied into an f32r tile (the BIR verifier requires
            f32r matmul operands to be produced with f32r rounding)."""
            row_r = rowp.tile([1, width], DT_R, tag="rowr", name="row_r")
            nc.scalar.copy(out=row_r, in_=row_f32)
            bp = psA.tile([P, width], mybir.dt.float32, tag="pmm",
                          name="bcast_ps")
            nc.tensor.matmul(bp, ones1r, row_r, start=True, stop=True)
            nc.vector.tensor_scalar_mul(sbuf_tile, bp, 1.0)
            return sbuf_tile

        # ---------------- stage B: ada_rms1 over the full batch -> h1T (bf16)
        poolBC = tc.alloc_tile_pool(name="poolBC", bufs=1)
        h1T = poolBC.tile([P, HC, T_FULL], DT, name="h1T")
        sB = tc.alloc_tile_pool(name="stB", bufs=2)

        FDB = 256  # stage-B tile width (keeps xt SBUF footprint low)

        def rms1_tile(nt):
            sl = slice(nt * FDB, (nt + 1) * FDB)
            xt = sB.tile([P, HC, FDB], mybir.dt.float32, tag="xt",
                         name="x_tile")
            nc.gpsimd.dma_start(out=xt, in_=xT_v[:, :, sl])
            ssum = psB.tile([1, FDB], mybir.dt.float32, tag="psmall",
                            name="ps_ss")
            for kc in range(HC):
                sq = sB.tile([P, FDB], DT_R, tag="sq", name="sq")
                nc.scalar.activation(sq, xt[:, kc, :], AF.Square)
                nc.tensor.matmul(ssum, ones_r, sq,
                                 start=(kc == 0), stop=(kc == HC - 1))
            rr = sB.tile([1, FDB], mybir.dt.float32, tag="rr", name="rr")
            nc.scalar.activation(rr, ssum, AF.Sqrt, bias=eps_t,
                                 scale=1.0 / HIDDEN)
            nc.vector.reciprocal(rr, rr)
            rrb = bcast_psum(rr, FDB,
                             sB.tile([P, FDB], mybir.dt.float32, tag="rrb",
                                     name="rrb"))
            for kc in range(HC):
                nc.vector.scalar_tensor_tensor(
                    out=h1T[:, kc, sl], in0=xt[:, kc, :],
                    scalar=g1[:, kc:kc + 1], in1=rrb,
                    op0=ALU.mult, op1=ALU.mult)

        rms1_tile(0)
        rms1_tile(1)

        # ---------------- stage C: QKV + RoPE (q blocks interleaved with
        # rms1 tiles so the PE streams matmuls while scalar runs Square/Sqrt)
        poolCD = tc.alloc_tile_pool(name="poolCD", bufs=1, side="right")
        qT = poolCD.tile([P, QC, T_OWN], DT, name="qT")
        kT = poolCD.tile([P, KC_HD, T_FULL], DT, name="kT")
        vtok = poolCD.tile([P, HC, HD], DT, name="vtok")
        sC = tc.alloc_tile_pool(name="stC", bufs=4, side="right")

        def q_block(nt):
            sl = slice(nt * FD, (nt + 1) * FD)
            for m in range(QC):
                strip = sC.tile([P, HC, P], DT, tag="w", name="wq_strip")
                nc.sync.dma_start(out=strip, in_=dram["wq"][m][:, :, :P])
                ps = psA.tile([P, FD], mybir.dt.float32, tag="pmm",
                              name="ps_q")
                for kc in range(HC):
                    nc.tensor.matmul(ps, strip[:, kc, :], h1T[:, kc, sl],
                                     start=(kc == 0), stop=(kc == HC - 1))
                nc.scalar.copy(out=qT[:, m, sl], in_=ps)

        q_block(0)
        rms1_tile(2)
        rms1_tile(3)
        q_block(1)
        for nt in range(4, 8):
            rms1_tile(nt)
        sB.release()

        for m in range(KC_HD):
            strip = sC.tile([P, HC, P], DT, tag="w", name="wk_strip")
            nc.sync.dma_start(out=strip, in_=dram["wk"][m][:, :, :P])
            for nt in range(NT_FULL):
                sl = slice(nt * FD, (nt + 1) * FD)
                ps = psA.tile([P, FD], mybir.dt.float32, tag="pmm",
                              name="ps_k")
                for kc in range(HC):
                    nc.tensor.matmul(ps, strip[:, kc, :], h1T[:, kc, sl],
                                     start=(kc == 0), stop=(kc == HC - 1))
                nc.scalar.copy(out=kT[:, m, sl], in_=ps)
        # RoPE (in-place on qT / kT), emitted BEFORE the v projection: the
        # vector-engine rope work overlaps the v matmuls on the PE.
        cos_f = sC.tile([P, T_FULL], DT, tag="cos", bufs=1, name="cos_f")
        sin_f = sC.tile([P, T_FULL], DT, tag="sin", bufs=1, name="sin_f")
        nc.gpsimd.dma_start(out=cos_f, in_=dram["cosT"][:, :])
        nc.gpsimd.dma_start(out=sin_f, in_=dram["sinT"][:, :])

        def rope_pair(u, v_, cos_t, sin_t, width):
            t1 = sC.tile([P, width], DT, tag="rt1", bufs=1, name="rope_t1")
            t2 = sC.tile([P, width], DT, tag="rt2", bufs=1, name="rope_t2")
            t3 = sC.tile([P, width], DT, tag="rt3", bufs=1, name="rope_t3")
            t4 = sC.tile([P, width], DT, tag="rt4", bufs=1, name="rope_t4")
            nc.vector.tensor_mul(t1, u, cos_t)
            nc.vector.tensor_mul(t2, u, sin_t)
            nc.vector.tensor_mul(t3, v_, sin_t)
            nc.vector.tensor_mul(t4, v_, cos_t)
            nc.vector.tensor_sub(u, t1, t3)
            nc.vector.tensor_add(v_, t4, t2)

        rope_pair(kT[:, 0, :], kT[:, 1, :], cos_f, sin_f, T_FULL)
        for h in range(NQ):
            rope_pair(qT[:, 2 * h, :], qT[:, 2 * h + 1, :],
                      cos_f[:, :T_OWN], sin_f[:, :T_OWN], T_OWN)

        # v token-major: [key-token-in-chunk, key-chunk, hd]
        wv_sb = sC.tile([P, HC, HD], DT, tag="wv", bufs=1, name="wv_sb")
        nc.sync.dma_start(out=wv_sb,
                          in_=dram["wv"][:, :, :].rearrange("c p d -> p c d"))
        for tm in range(T_FULL // P):
            ps = psA.tile([P, HD], mybir.dt.float32, tag="pmm",
                          name="ps_v")
            tsl = slice(tm * P, (tm + 1) * P)
            for kc in range(HC):
                nc.tensor.matmul(ps, h1T[:, kc, tsl], wv_sb[:, kc, :],
                                 start=(kc == 0), stop=(kc == HC - 1))
            nc.scalar.copy(out=vtok[:, tm, :], in_=ps)
        sC.release()
        poolBC.release()

        # ---------------- stage D: attention (column softmax, no transposes)
        # Scores are emitted two heads ahead of the softmax/ctx consumer so
        # each head's Exp pass (scalar engine) has ~2 head-times of runway
        # instead of being just-in-time (which stalled the PE and tripped
        # the HAM clock throttle).
        poolDE = tc.alloc_tile_pool(name="poolDE", bufs=1)
        ctxT = poolDE.tile([P, QC, T_OWN], DT, name="ctxT")
        with tc.tile_pool(name="stD", bufs=3) as sD:
            attn_tiles = {}

            def scores_block(h):
                attnT = sD.tile([P, HC, T_OWN], DT, tag="attn", name="attnT")
                attn_tiles[h] = attnT
                for sm in range(T_FULL // P):
                    for nt in range(NT_OWN):
                        sl = slice(nt * FD, (nt + 1) * FD)
                        ps = psA.tile([P, FD], mybir.dt.float32, tag="pmm",
                                      name="ps_sc")
                        for dc in range(KC_HD):
                            nc.tensor.matmul(
                                ps, kT[:, dc, sm * P:(sm + 1) * P],
                                qT[:, 2 * h + dc, sl],
                                start=(dc == 0), stop=(dc == KC_HD - 1))
                        nc.scalar.activation(attnT[:, sm, sl], ps, AF.Exp,
                                             scale=1.0 / 16.0)

            def softmax_ctx_block(h):
                attnT = attn_tiles.pop(h)
                rec = sD.tile([1, T_OWN], mybir.dt.float32, tag="rec",
                              name="rec")
                for nt in range(NT_OWN):
                    sl = slice(nt * FD, (nt + 1) * FD)
                    cs = psB.tile([1, FD], mybir.dt.float32, tag="psmall",
                                  name="ps_cs")
                    for kc in range(HC):
                        nc.tensor.matmul(cs, ones_bf, attnT[:, kc, sl],
                                         start=(kc == 0), stop=(kc == HC - 1))
                    nc.vector.reciprocal(rec[:, sl], cs)
                for nt in range(NT_OWN):
                    sl = slice(nt * FD, (nt + 1) * FD)
                    recb = bcast_psum(rec[:, sl], FD,
                                      sD.tile([P, FD], mybir.dt.float32,
                                              tag="recb", name="recb"))
                    for dm in range(KC_HD):
                        ps = psA.tile([P, FD], mybir.dt.float32, tag="pmm",
                                      name="ps_ctx")
                        for kc in range(HC):
                            nc.tensor.matmul(
                                ps, vtok[:, kc, dm * P:(dm + 1) * P],
                                attnT[:, kc, sl],
                                start=(kc == 0), stop=(kc == HC - 1))
                        nc.vector.tensor_mul(ctxT[:, 2 * h + dm, sl], ps,
                                             recb)

            scores_block(0)
            scores_block(1)
            for h in range(NQ):
                if h + 2 < NQ:
                    scores_block(h + 2)
                softmax_ctx_block(h)
        poolCD.release()

        # ---------------- stage E: o_proj + residual + ada_rms2 -> h2T
        # nt-major: rms2 of tile 0 overlaps o-proj matmuls of tile 1.
        poolEF = tc.alloc_tile_pool(name="poolEF", bufs=1, side="right")
        h2T = poolEF.tile([P, HC, T_OWN], DT, name="h2T")
        with tc.tile_pool(name="stE", bufs=2) as sE:
            # The rms2 sum-of-squares is fused into the o-proj loop (Square +
            # interleaved psB accumulation per m-chunk), and the sqrt/recip/
            # broadcast tail of tile 0 is emitted two m-groups into the
            # second o-proj pass so its cross-engine chain runs under o-proj
            # matmuls instead of stalling the PE before stage F.
            res2_tiles = []
            ssums = []

            def o_proj_group(nt, m, res2, ssum):
                sl = slice(nt * FD, (nt + 1) * FD)
                strip = sE.tile([P, QC, P], DT, tag="w", bufs=4,
                                name="wo_strip")
                nc.sync.dma_start(out=strip, in_=dram["wo"][m][:, :, :P])
                ps = psA.tile([P, FD], mybir.dt.float32, tag="pmm",
                              name="ps_o")
                for kc in range(QC):
                    nc.tensor.matmul(ps, strip[:, kc, :], ctxT[:, kc, sl],
                                     start=(kc == 0), stop=(kc == QC - 1))
                xo = sE.tile([P, FD], mybir.dt.float32, tag="xo", bufs=4,
                             name="xo")
                nc.gpsimd.dma_start(out=xo, in_=xT_v[:, m, sl])
                nc.vector.scalar_tensor_tensor(
                    out=res2[:, m, :], in0=ps, scalar=0.0,
                    in1=xo, op0=ALU.bypass, op1=ALU.add)
                sq = sE.tile([P, FD], DT_R, tag="sq", name="sq2")
                nc.scalar.activation(sq, res2[:, m, :], AF.Square)
                nc.tensor.matmul(ssum, ones_r, sq,
                                 start=(m == 0), stop=(m == HC - 1))

            def rms2_tail(nt):
                sl = slice(nt * FD, (nt + 1) * FD)
                res2 = res2_tiles[nt]
                rr = sE.tile([1, FD], mybir.dt.float32, tag="rr", name="rr2")
                nc.scalar.activation(rr, ssums[nt], AF.Sqrt, bias=eps_t,
                                     scale=1.0 / HIDDEN)
                nc.vector.reciprocal(rr, rr)
                rrb = bcast_psum(rr, FD,
                                 sE.tile([P, FD], mybir.dt.float32,
                                         tag="rrb", name="rrb2"))
                for kc in range(HC):
                    nc.vector.scalar_tensor_tensor(
                        out=h2T[:, kc, sl], in0=res2[:, kc, :],
                        scalar=g2[:, kc:kc + 1], in1=rrb,
                        op0=ALU.mult, op1=ALU.mult)

            for nt in range(NT_OWN):
                res2_tiles.append(sE.tile([P, HC, FD], mybir.dt.float32,
                                          tag="res2", name="res2"))
                ssums.append(psB.tile([1, FD], mybir.dt.float32,
                                      tag="psmall", name="ps_ss2"))
            for m in range(HC):
                o_proj_group(0, m, res2_tiles[0], ssums[0])
            nc.gpsimd.dma_start(out=res2T_v[:, :, 0:FD], in_=res2_tiles[0])
            for m in range(HC):
                o_proj_group(1, m, res2_tiles[1], ssums[1])
                if m == 1:
                    rms2_tail(0)
            nc.gpsimd.dma_start(out=res2T_v[:, :, FD:T_OWN], in_=res2_tiles[1])
            rms2_tail(1)
        poolDE.release()

        # ---------------- stage F: SwiGLU MLP + final residual
        with tc.tile_pool(name="stF", bufs=2) as sF:
            for tt in range(NT_OWN):
                sl = slice(tt * FD, (tt + 1) * FD)
                act = sF.tile([P, IC, FD], DT, tag="act", bufs=1, name="act")
                for im in range(IC):
                    gstrip = sF.tile([P, HC, P], DT, tag="w", bufs=4,
                                     name="wg_strip")
                    nc.sync.dma_start(out=gstrip, in_=dram["wg"][im][:, :, :P])
                    ps_g = psA.tile([P, FD], mybir.dt.float32, tag="pmm",
                                    name="ps_g")
                    for kc in range(HC):
                        nc.tensor.matmul(ps_g, gstrip[:, kc, :],
                                         h2T[:, kc, sl],
                                         start=(kc == 0), stop=(kc == HC - 1))
                    sil = sF.tile([P, FD], mybir.dt.float32, tag="sil",
                                  name="sil")
                    nc.scalar.activation(sil, ps_g, AF.Sigmoid)
                    nc.vector.tensor_mul(sil, sil, ps_g)
                    ustrip = sF.tile([P, HC, P], DT, tag="w", bufs=4,
                                     name="wu_strip")
                    nc.sync.dma_start(out=ustrip, in_=dram["wu"][im][:, :, :P])
                    ps_u = psA.tile([P, FD], mybir.dt.float32, tag="pmm",
                                    name="ps_u")
                    for kc in range(HC):
                        nc.tensor.matmul(ps_u, ustrip[:, kc, :],
                                         h2T[:, kc, sl],
                                         start=(kc == 0), stop=(kc == HC - 1))
                    nc.vector.tensor_mul(act[:, im, :], sil, ps_u)
                for dm in range(HC):
                    dstrip = sF.tile([P, IC, P], DT, tag="wd", bufs=2,
                                     name="wd_strip")
                    nc.sync.dma_start(out=dstrip, in_=dram["wd"][dm][:, :, :P])
                    ps_d = psA.tile([P, FD], mybir.dt.float32, tag="pmm",
                                    name="ps_d")
                    for kc in range(IC):
                        nc.tensor.matmul(ps_d, dstrip[:, kc, :],
                                         act[:, kc, :],
                                         start=(kc == 0), stop=(kc == IC - 1))
                    r2c = sF.tile([P, FD], mybir.dt.float32, tag="r2c",
                                  name="r2c")
                    nc.gpsimd.dma_start(out=r2c, in_=res2T_v[:, dm, sl])
                    ot = sF.tile([P, FD], mybir.dt.float32, tag="ot",
                                 name="ot")
                    nc.vector.tensor_add(ot, ps_d, r2c)
                    nc.gpsimd.dma_start(out=outT_v[:, dm, sl], in_=ot)
        poolEF.release()


def _prep_inputs(x, pos_ids, time_emb, ln1_w, ln1_tw, ln2_w, ln2_tw,
                 Wq, Wk, Wv, Wo, Wg, Wu, Wd):
    """Host-side layout prep. Returns list of per-core in_maps."""
    shared = {
        "wq": _strips(Wq.T, HC, QC),
        "wk": _strips(Wk.T, HC, KC_HD),
        "wv": np.ascontiguousarray(Wv.T.reshape(HC, P, HD)).astype(BF16),
        "wo": _strips(Wo.T, QC, HC),
        "wg": _strips(Wg.T, HC, IC),
        "wu": _strips(Wu.T, HC, IC),
        "wd": _strips(Wd.T, IC, HC),
    }
    inv_freq = 1.0 / (ROPE_BASE **
                      (np.arange(0, HD, 2, dtype=np.float64) / HD))
    in_maps = []
    for c in range(N_CORES):
        b, half = c // 2, c % 2
        perm = np.r_[np.arange(half * T_OWN, (half + 1) * T_OWN),
                     np.arange((1 - half) * T_OWN, (2 - half) * T_OWN)]
        xTb = np.ascontiguousarray(np.asarray(x[b]).T[:, perm]).astype(F32)
        ang = (np.asarray(pos_ids[b])[perm].astype(np.float64)[:, None]
               * inv_freq[None, :])
        tb = np.asarray(time_emb[b]).astype(np.float64)
        g1 = np.asarray(ln1_w) * (1.0 + tb @ np.asarray(ln1_tw, np.float64).T)
        g2 = np.asarray(ln2_w) * (1.0 + tb @ np.asarray(ln2_tw, np.float64).T)
        m = dict(shared)
        m["xT"] = xTb
        m["cosT"] = np.ascontiguousarray(np.cos(ang).T).astype(BF16)
        m["sinT"] = np.ascontiguousarray(np.sin(ang).T).astype(BF16)
        m["g1"] = np.ascontiguousarray(
            np.asarray(g1, np.float32).reshape(HC, P).T)
        m["g2"] = np.ascontiguousarray(
            np.asarray(g2, np.float32).reshape(HC, P).T)
        in_maps.append(m)
    return in_maps


def kernel(**inputs):
    global LAST_RESULTS
    from concourse.bass_utils import run_bass_kernel_spmd

    nc = build_program()
    in_maps = _prep_inputs(**{k: np.asarray(v) for k, v in inputs.items()})
    trace = bool(int(os.environ.get("KERNEL_TRACE", "0")))
    kw = {}
    if os.environ.get("KERNEL_TMPDIR"):
        os.makedirs(os.environ["KERNEL_TMPDIR"], exist_ok=True)
        kw["tmpdir"] = os.environ["KERNEL_TMPDIR"]
    res = run_bass_kernel_spmd(nc, in_maps, core_ids=list(range(N_CORES)),
                               trace=trace, **kw)
    LAST_RESULTS = res
    out = np.empty((B, S, HIDDEN), dtype=F32)
    for c in range(N_CORES):
        b, half = c // 2, c % 2
        out[b, half * T_OWN:(half + 1) * T_OWN, :] = res.results[c]["outT"].T
    return out
